# revision 1
# baseline (speedup 1.0000x reference)
"""Causal self-attention (softmax over the QUERY axis) for Trainium2, 8 cores.

Reference semantics (B=2, S=2048, D=1024, H=16, HD=64):
    q = x @ Wq; k = x @ Wk; v = x @ Wv          (per batch)
    s[b,h,q,k] = <q_bqh, k_bkh>;  mask k > q -> -inf
    w = softmax(s / sqrt(1024), axis=q)          # normalize over QUERY axis
    ctx[b,q,h,:] = sum_k w[b,h,q,k] * v[b,k,h,:]

Sharding: core c handles batch b = c // 4 and head group g = c % 4
(4 heads: 4g..4g+3).  Per core everything is done in a transposed
score layout S^T[k, q], which makes the query-axis softmax a FREE-AXIS
reduction, and the 1/Z[k] normalizer folds into V rows (no per-element
divide): ctx[q,d] = sum_k exp(s)/Z[k] * v[k,d] = sum_k exp(s) * (v[k,d]/Z[k]).

Device layouts (per core):
    xT  [1024, 2048] bf16 (host-transposed)  -> SBUF [128, 8, 2048]
    Wq/Wk/Wv column slices [1024, 256] bf16  -> SBUF [128, 8, 256]
    qT/kT  [128(2 heads x 64), 2 pairs, 2048] bf16 (projection output)
    v      [128(s in tile), 16 kt, 256(4 heads x 64)] bf16
    E      packed exp(scores^T): row kt occupies cols [off_kt, off_kt+2048-128kt)
    out    [256(4 heads x 64), 2048] f32 = ctx^T; host transposes back.
"""

import numpy as np
import ml_dtypes
from contextlib import ExitStack

import concourse.bass as bass
import concourse.tile as tile
from concourse import bacc, mybir
from concourse.bass_utils import run_bass_kernel_spmd

BF16 = mybir.dt.bfloat16
F32 = mybir.dt.float32

B, S, D, H, HD = 2, 2048, 1024, 16, 64
NCORES = 8
HL = 4                       # heads per core
KC = D // 128                # 8 contraction chunks
KT = S // 128                # 16 key tiles
QC = S // 512                # 4 query chunks of 512
SCALE = 1.0 / float(np.sqrt(np.float32(D)))   # 1/32

W_ROW = [S - 128 * kt for kt in range(KT)]          # valid width of E row kt
E_OFF = np.concatenate([[0], np.cumsum(W_ROW)]).astype(int)
E_TOT = int(E_OFF[-1])                              # 17408



def _emit(ctx: ExitStack, tc: tile.TileContext, out_ap, xT, wq, wk, wv):
    nc = tc.nc
    Exp = mybir.ActivationFunctionType.Exp

    consts = ctx.enter_context(tc.tile_pool(name="consts", bufs=1))
    qkp = ctx.enter_context(tc.tile_pool(name="qk", bufs=1))
    vp = ctx.enter_context(tc.tile_pool(name="v", bufs=1))
    epool = ctx.enter_context(tc.tile_pool(name="e", bufs=2))
    zpool = ctx.enter_context(tc.tile_pool(name="z", bufs=4))
    spool = ctx.enter_context(tc.tile_pool(name="scr", bufs=4))
    outp = ctx.enter_context(tc.tile_pool(name="outp", bufs=1))
    # scores rows: [128, 1536] = 3 banks x 2 bufs = 6 banks; projections and
    # ctx accumulations share one 2-slot [*, 512] pool (2 banks).
    sc_ps = ctx.enter_context(tc.tile_pool(name="sc_ps", bufs=2, space="PSUM"))
    small_ps = ctx.enter_context(tc.tile_pool(name="small_ps", bufs=2, space="PSUM"))

    # ---- loads: weights on the SP HWDGE ring, xT chunks on the ACT ring
    # (chunk 3 first: score rows are emitted descending) ----
    w_sb = {}
    for name, t in (("q", wq), ("k", wk), ("v", wv)):
        w_sb[name] = consts.tile([128, KC, HL * HD], BF16, tag=f"w{name}",
                                 name=f"w{name}_sb")
        nc.sync.dma_start(out=w_sb[name], in_=t.rearrange("(c p) n -> p c n", p=128))
    xT_r = xT.rearrange("(c p) s -> p c s", p=128)
    xT_cs = [None] * 4
    for sc in (3, 2, 1, 0):
        xT_cs[sc] = consts.tile([128, KC, 512], BF16, tag=f"xT{sc}",
                                name=f"xT{sc}_sb")
        nc.scalar.dma_start(out=xT_cs[sc],
                            in_=xT_r[:, :, 512 * sc:512 * sc + 512])

    def xT_slice(c, lo, w):
        sc, o = divmod(lo, 512)
        assert o + w <= 512
        return xT_cs[sc][:, c, o:o + w]

    qT_sb = qkp.tile([128, 2, S], BF16, tag="qT")
    kT_sb = qkp.tile([128, 2, S], BF16, tag="kT")
    v_sb = vp.tile([128, KT, HL * HD], BF16, tag="v")
    v2_sb = vp.tile([128, KT, HL * HD], BF16, tag="v2")
    out_sb = outp.tile([128, 2, S], F32, tag="out")

    def proj_chain(name, pair, qc):
        dst = qT_sb if name == "q" else kT_sb
        ps = small_ps.tile([128, 512], F32, tag="ps512", name="pps")
        for c in range(KC):
            nc.tensor.matmul(
                ps,
                w_sb[name][:, c, 128 * pair:128 * pair + 128],
                xT_cs[qc][:, c, :],
                start=(c == 0), stop=(c == KC - 1),
            )
        nc.vector.tensor_copy(dst[:, pair, 512 * qc:512 * qc + 512], ps)

    def proj_v():
        # v natural layout: out partitions = s-within-tile, cols = 4 heads x 64
        for st in range(KT):
            ps = small_ps.tile([128, HL * HD], F32, tag="ps512", name="pps")
            for c in range(KC):
                nc.tensor.matmul(
                    ps,
                    xT_slice(c, 128 * st, 128),
                    w_sb["v"][:, c, :],
                    start=(c == 0), stop=(c == KC - 1),
                )
            nc.vector.tensor_copy(v_sb[:, st, :], ps)

    def alloc_head(h):
        zp = zpool.tile([128, KT, 2], F32, tag="zp", name=f"zp{h}")
        inv = zpool.tile([128, KT], F32, tag="inv", name=f"inv{h}")
        nc.vector.memset(zp, 0.0)
        nc.vector.memset(inv, 0.0)
        return {"zp": zp, "inv": inv, "e": [None] * KT, "h": h}

    def score_row(st, kt):
        """scores^T row kt for head st['h']: matmuls + exp(+Z accum) + diag fix."""
        h = st["h"]
        pair, half = divmod(h, 2)
        pb = 64 * half
        q0k = 128 * kt
        W = S - q0k
        # rows 4..15 get a third slot so the next pair's score rows never
        # wait on ctx chains releasing E (rows 0..3 are too big to afford
        # a third copy, but they are also the last ones the next head
        # reaches, by which point the ctx chains have freed them).
        e_row = epool.tile([128, W], BF16, tag=f"E{kt}", name=f"e{kt}",
                           bufs=(3 if kt >= 4 else 2))
        st["e"][kt] = e_row
        lhsT = kT_sb[pb:pb + 64, pair, q0k:q0k + 128]   # [64, 128]
        tiles = [(q0k, min(W, 1536))]
        if W > 1536:
            tiles.append((q0k + 1536, W - 1536))
        dve_z = kt >= 8    # short rows: Z via DVE post-zero sum (ACT stays hot)
        for ti, (lo, w) in enumerate(tiles):
            ps = sc_ps.tile([128, w], F32, tag="sc", name="scps")
            c0 = 0
            while c0 < w:
                c1 = min(w, c0 + 512)
                nc.tensor.matmul(
                    ps[:, c0:c1],
                    lhsT,
                    qT_sb[pb:pb + 64, pair, lo + c0:lo + c1],
                    start=True, stop=True,
                )
                c0 = c1
            if dve_z:
                nc.scalar.activation(
                    e_row[:, lo - q0k:lo - q0k + w], ps[:, 0:w],
                    Exp, scale=SCALE,
                )
            else:
                nc.scalar.activation(
                    e_row[:, lo - q0k:lo - q0k + w], ps[:, 0:w],
                    Exp, scale=SCALE,
                    accum_out=st["zp"][:, kt, ti:ti + 1],
                )
        # diagonal block: cols [0, 128) hold q in [128kt, 128kt+128);
        # entries with q < k (j < p) are invalid.
        diag = e_row[:, 0:128]
        if not dve_z:
            # gather the invalid part (its sum is subtracted from Z);
            # is_lt is unimplemented in walrus codegen, so use is_ge with
            # negated affine coefficients (j < p <=> p - j - 1 >= 0).
            scr = spool.tile([128, 128], BF16, tag="scr", name="scr")
            nc.gpsimd.affine_select(
                scr, diag, pattern=[[-1, 128]],
                compare_op=mybir.AluOpType.is_ge, fill=0.0,
                base=-1, channel_multiplier=1,
            )
            nc.vector.tensor_reduce(
                st["inv"][:, kt:kt + 1], scr,
                axis=mybir.AxisListType.X, op=mybir.AluOpType.add,
            )
        nc.gpsimd.affine_select(
            diag, diag, pattern=[[1, 128]],
            compare_op=mybir.AluOpType.is_ge, fill=0.0,
            base=0, channel_multiplier=-1,
        )
        if dve_z:
            # post-zero row sum is exactly the valid Z contribution
            nc.vector.tensor_reduce(
                st["zp"][:, kt, 0:1], e_row[:, 0:W],
                axis=mybir.AxisListType.X, op=mybir.AluOpType.add,
            )

    def z_v2(st, k0, k1):
        """finalize Z for rows [k0, k1) and scale V rows by 1/Z."""
        h = st["h"]
        n = k1 - k0
        zs = zpool.tile([128, n], F32, tag="zs", name="zs")
        nc.vector.tensor_reduce(zs, st["zp"][:, k0:k1, :],
                                axis=mybir.AxisListType.X,
                                op=mybir.AluOpType.add)
        zv = zpool.tile([128, n], F32, tag="zv", name="zv")
        nc.vector.tensor_sub(zv, zs, st["inv"][:, k0:k1])
        zi = zpool.tile([128, n], F32, tag="zi", name="zi")
        nc.vector.reciprocal(zi, zv)
        zia = zi[:, :]
        zi_bc = bass.AP(tensor=zia.tensor, offset=zia.offset,
                        ap=[zia.ap[0], zia.ap[1], [0, HD]])
        nc.vector.tensor_mul(
            v2_sb[:, k0:k1, HD * h:HD * h + HD],
            v_sb[:, k0:k1, HD * h:HD * h + HD],
            zi_bc,
        )

    def ctx_chain(st, qc):
        """one solo ctx^T accumulation chain for (head, qc) + copy to out_sb."""
        h = st["h"]
        pair, half = divmod(h, 2)
        ps = small_ps.tile([64, 512], F32, tag="ps512", name="cps")
        n_kt = 4 * qc + 4
        for kt in range(n_kt):
            q0 = max(512 * qc, 128 * kt)
            w = 512 * qc + 512 - q0
            rhs = st["e"][kt][:, q0 - 128 * kt:q0 - 128 * kt + w]
            nc.tensor.matmul(
                ps[:, q0 - 512 * qc:512],
                v2_sb[:, kt, HD * h:HD * h + HD],
                rhs,
                start=(kt == 0), stop=(kt == n_kt - 1),
            )
        nc.vector.tensor_copy(
            out_sb[64 * half:64 * half + 64, pair, 512 * qc:512 * qc + 512], ps)

    def out_dma(pair, qc):
        nc.sync.dma_start(
            out=out_ap[128 * pair:128 * pair + 128, 512 * qc:512 * qc + 512],
            in_=out_sb[:, pair, 512 * qc:512 * qc + 512],
        )

    def ctx_pair_packed(sta, stb, qc):
        """col-packed ctx chains for a whole pair (heads sta, stb) at qc."""
        pair = sta["h"] // 2
        ps = small_ps.tile([128, 512], F32, tag="ps512", name="cpp")
        n_kt = 4 * qc + 4
        for kt in range(n_kt):
            q0 = max(512 * qc, 128 * kt)
            w = 512 * qc + 512 - q0
            for half, st in ((0, sta), (1, stb)):
                h = st["h"]
                rhs = st["e"][kt][:, q0 - 128 * kt:q0 - 128 * kt + w]
                nc.tensor.matmul(
                    ps[64 * half:64 * half + 64, q0 - 512 * qc:512],
                    v2_sb[:, kt, HD * h:HD * h + HD],
                    rhs,
                    start=(kt == 0), stop=(kt == n_kt - 1),
                    tile_position=(0, 64 * half),
                    skip_group_check=True,
                )
        nc.vector.tensor_copy(out_sb[:, pair, 512 * qc:512 * qc + 512], ps)

    # ---- emission (order = scheduling priority; heads' score rows always
    # outrank filler work so head transitions have no priority bubble) ----
    st0 = alloc_head(0)
    for qc in (3, 2, 1, 0):           # head 0 interleaved with its projections
        proj_chain("q", 0, qc)
        proj_chain("k", 0, qc)
        for kt in range(4 * qc + 3, 4 * qc - 1, -1):
            score_row(st0, kt)
    st1 = alloc_head(1)
    for kt in range(KT - 1, -1, -1):  # head 1 rows outrank all filler
        score_row(st1, kt)
    proj_v()                          # filler during heads 0-1 exp waits
    z_v2(st0, 0, KT)                  # (after proj_v: v_sb RAW order)
    z_v2(st1, 0, KT)
    for qc in (3, 2, 1, 0):           # pair-1 projections: filler
        proj_chain("q", 1, qc)
        proj_chain("k", 1, qc)
    st2 = alloc_head(2)
    for kt in range(KT - 1, -1, -1):  # E slots: rows 4-15 have a 3rd slot;
        score_row(st2, kt)            # rows 0-3 wait on the chain below
    ctx_pair_packed(st0, st1, 0)      # frees pair-0's E rows 0-3 early
    out_dma(0, 0)
    z_v2(st2, 0, KT)
    # head 3: ascending rows, per-group Z; overlaps head 2 on ACT since its
    # E slots are already free (3rd slot / chain-0 release)
    st3 = alloc_head(3)
    for g in range(4):
        for kt in range(4 * g, 4 * g + 4):
            score_row(st3, kt)
        z_v2(st3, 4 * g, 4 * g + 4)
        if g >= 1:                    # rest of pair-0 ctx: fills PE slack
            ctx_pair_packed(st0, st1, g)
            out_dma(0, g)
    for g in range(4):                # pair-1 ctx: packed, progressive
        ctx_pair_packed(st2, st3, g)
        out_dma(1, g)


_PROG = None


def _build_program():
    global _PROG
    if _PROG is not None:
        return _PROG
    nc = bacc.Bacc("TRN2", target_bir_lowering=False, debug=False,
                   num_devices=NCORES)
    xT = nc.dram_tensor("xT", [D, S], BF16, kind="ExternalInput").ap()
    wq = nc.dram_tensor("wq", [D, HL * HD], BF16, kind="ExternalInput").ap()
    wk = nc.dram_tensor("wk", [D, HL * HD], BF16, kind="ExternalInput").ap()
    wv = nc.dram_tensor("wv", [D, HL * HD], BF16, kind="ExternalInput").ap()
    out = nc.dram_tensor("out", [HL * HD, S], F32, kind="ExternalOutput").ap()
    with tile.TileContext(nc) as tc:
        with ExitStack() as stack:
            _emit(stack, tc, out, xT, wq, wk, wv)
    nc.compile()
    _PROG = nc
    return nc


def make_in_maps(x, Wq, Wk, Wv):
    bf = ml_dtypes.bfloat16
    in_maps = []
    for core in range(NCORES):
        b, g = divmod(core, NCORES // B)
        cols = slice(HL * HD * g, HL * HD * (g + 1))
        in_maps.append({
            "xT": np.ascontiguousarray(np.asarray(x[b]).T).astype(bf),
            "wq": np.ascontiguousarray(np.asarray(Wq)[:, cols]).astype(bf),
            "wk": np.ascontiguousarray(np.asarray(Wk)[:, cols]).astype(bf),
            "wv": np.ascontiguousarray(np.asarray(Wv)[:, cols]).astype(bf),
        })
    return in_maps


def assemble(results):
    out = np.empty((B, S, H * HD), np.float32)
    for core in range(NCORES):
        b, g = divmod(core, NCORES // B)
        out[b, :, HL * HD * g:HL * HD * (g + 1)] = results[core]["out"].T
    return out


def kernel(**inputs):
    nc = _build_program()
    in_maps = make_in_maps(inputs["x"], inputs["Wq"], inputs["Wk"], inputs["Wv"])
    res = run_bass_kernel_spmd(nc, in_maps, list(range(NCORES)))
    return assemble(res.results)

